# revision 23
# baseline (speedup 1.0000x reference)
"""Trainium2 Bass kernel for CustomGPT2Attention (B=4, S=2048, D=1024, H=16).

Strategy: tensor-parallel over heads. Each of the 8 NeuronCores owns 2 heads
(a 128-wide slice of the QKV projections and the matching 128 rows of Wo),
computes its partial output projection over the full batch, and the host sums
the 8 partials (the "all-reduce" of the row-parallel c_proj) plus bo.

All on-device layouts are chosen so no transposes are ever needed:
  - host ships hidden^T [D, B*S] (bf16)
  - QT, KT come out of the projection as [hd_local, B*S]
  - scores are computed transposed, ST[kv, q] = K^T Q, softmax runs along
    the partition (kv) axis; the denominators come for free from a 65th
    ones-column appended to each AV stationary (V tile), so no separate
    denominator matmuls are needed
  - AV output OT[hd_local, q] is directly the lhsT of the output projection
  - the output projection result is DMA'd straight from PSUM to DRAM
Compute dtype bf16 (fp32 PSUM accumulation everywhere).
"""

import collections
import os
import sys

for _p in ("/opt/trn_rl_repo", "/root/.axon_site/_ro/trn_rl_repo"):
    if os.path.isdir(_p) and _p not in sys.path:
        sys.path.insert(0, _p)

import numpy as np
import ml_dtypes

BF16 = ml_dtypes.bfloat16

B, S, D, H, HD = 4, 2048, 1024, 16, 64
BS = B * S            # 8192 tokens
NCORES = 8
DL = D // NCORES      # 128 = per-core slice (2 heads x 64)
NK = D // 128         # 8 contraction chunks for the projections
SQ = 512              # q free-block width
NJ = S // SQ          # 4 q-blocks per batch
NT = BS // 128        # 64 s-tiles (of 128 tokens)
SCALE = 1.0 / 8.0     # 1/sqrt(HD)
VW = 130              # v_sb per-tile width: [vA(64) | 1 | vB(64) | 1]

_CACHE = {}
LAST_RESULTS = None
KDEBUG = bool(os.environ.get("KDEBUG"))


def _build_nc():
    import concourse.bacc as bacc
    import concourse.tile as tile
    import concourse.mybir as mybir
    import bass_rust

    dt = mybir.dt
    AF = mybir.ActivationFunctionType

    class _Bacc(bacc.Bacc):
        # All ACT functions we use (Exp, Ln) live in the
        # natural_log_exp_and_others table set. The stock table-load pass
        # assigns each function its first matching set, which thrashes
        # ACT_TABLE_LOADs (~1.3us each) between exp/ln sets inside the
        # softmax loop. Restrict the pass to the one set that has them all.
        def insert_act_table_loads(self):
            from concourse.hw_specs import get_activation_tables
            has_activation = any(
                isinstance(i, mybir.InstActivation)
                for b in self.main_func.blocks
                for i in b.instructions
            )
            if not has_activation:
                return
            tables = []
            for name, funcs in get_activation_tables(self.m.arch).items():
                if name != "natural_log_exp_and_others":
                    funcs = set()
                tables.append((name, funcs))
            bass_rust.insert_act_table_loads(self, tables)

    nc = _Bacc(
        "TRN2", target_bir_lowering=False, debug=False, num_devices=NCORES
    )

    ht_d = nc.dram_tensor("ht", [D, BS], dt.bfloat16, kind="ExternalInput").ap()
    wq_d = nc.dram_tensor("wq", [D, DL], dt.bfloat16, kind="ExternalInput").ap()
    wk_d = nc.dram_tensor("wk", [D, DL], dt.bfloat16, kind="ExternalInput").ap()
    wv_d = nc.dram_tensor("wv", [D, DL], dt.bfloat16, kind="ExternalInput").ap()
    wo_d = nc.dram_tensor("wo", [DL, D], dt.bfloat16, kind="ExternalInput").ap()
    bq_d = nc.dram_tensor("bq", [DL, 1], dt.float32, kind="ExternalInput").ap()
    bk_d = nc.dram_tensor("bk", [DL, 1], dt.float32, kind="ExternalInput").ap()
    bvb_d = nc.dram_tensor("bvb", [DL, 1], dt.float32, kind="ExternalInput").ap()
    id_d = nc.dram_tensor("ident", [128, 128], dt.bfloat16, kind="ExternalInput").ap()
    mk_d = nc.dram_tensor("mask", [128, 4 * 1024], dt.bfloat16, kind="ExternalInput").ap()
    out_d = nc.dram_tensor("out", [BS, D], dt.bfloat16, kind="ExternalOutput").ap()

    taps = None
    if KDEBUG:
        taps = {
            "dqt": nc.dram_tensor("dqt", [128, BS], dt.bfloat16, kind="ExternalOutput").ap(),
            "dkt": nc.dram_tensor("dkt", [128, BS], dt.bfloat16, kind="ExternalOutput").ap(),
            "dv": nc.dram_tensor("dv", [128, VW * NT], dt.bfloat16, kind="ExternalOutput").ap(),
            "dot": nc.dram_tensor("dot", [128, BS], dt.bfloat16, kind="ExternalOutput").ap(),
            "dav": nc.dram_tensor("dav", [128, 1024], dt.float32, kind="ExternalOutput").ap(),
            "drc": nc.dram_tensor("drc", [128, 1024], dt.bfloat16, kind="ExternalOutput").ap(),
            "dbc": nc.dram_tensor("dbc", [128, 1024], dt.float32, kind="ExternalOutput").ap(),
            "dpt": nc.dram_tensor("dpt", [128, 1024], dt.bfloat16, kind="ExternalOutput").ap(),
        }

    with tile.TileContext(nc) as tc:
        _body(tc, nc, mybir, ht_d, wq_d, wk_d, wv_d, wo_d, bq_d, bk_d, bvb_d,
              id_d, mk_d, out_d, taps)

    nc.compile()
    return nc


def _body(tc, nc, mybir, ht_d, wq_d, wk_d, wv_d, wo_d, bq_d, bk_d, bvb_d,
          id_d, mk_d, out_d, taps=None):
    from contextlib import ExitStack

    dt = mybir.dt
    AF = mybir.ActivationFunctionType

    ctx = ExitStack()
    with ctx:
        consts = ctx.enter_context(tc.tile_pool(name="consts", bufs=1))

        # --- constants / weights (persist whole kernel) ---
        wq_sb = consts.tile([128, D], dt.bfloat16, name="wq_sb")
        wk_sb = consts.tile([128, D], dt.bfloat16, name="wk_sb")
        wv_sb = consts.tile([128, D], dt.bfloat16, name="wv_sb")
        wo_sb = consts.tile([128, D], dt.bfloat16, name="wo_sb")
        bq_sb = consts.tile([128, 1], dt.float32, name="bq_sb")
        bk_sb = consts.tile([128, 1], dt.float32, name="bk_sb")
        bvb_sb = consts.tile([128, 1], dt.float32, name="bvb_sb")
        id_sb = consts.tile([128, 128], dt.bfloat16, name="id_sb")
        mask_sb = consts.tile([128, 4 * 1024], dt.bfloat16, name="mask_sb")
        ones_row = consts.tile([128, 64], dt.bfloat16, name="ones_row")

        # weights are [D, DL] in DRAM; load as 8 lhsT tiles [128, 128] side by
        # side -> SBUF [128, 8*128]
        def load_w(w_d, w_sb):
            nc.sync.dma_start(
                w_sb.rearrange("p (k n) -> p k n", k=NK),
                w_d.rearrange("(k p) n -> p k n", p=128),
            )
        nc.gpsimd.memset(ones_row[:, :], 1.0)

        # --- persistent activation tensors ---
        qt_sb = consts.tile([128, BS], dt.bfloat16, name="qt_sb")   # Q^T
        kt_sb = consts.tile([128, BS], dt.bfloat16, name="kt_sb")   # K^T
        vt_sb = consts.tile([128, BS], dt.bfloat16, name="vt_sb")   # V^T
        # V with a ones column per head: s-tile t occupies cols
        # [t*VW, (t+1)*VW) as [vA(64) | 1 | vB(64) | 1]; the AV stationaries
        # are the contiguous 65-col slices [0:65] and [65:130].
        v_sb = consts.tile([128, VW * NT], dt.bfloat16, name="v_sb")
        ot_sb = consts.tile([128, BS], dt.bfloat16, name="ot_sb")   # attn out ^T

        # ones columns (col 64 of each 65-wide half-tile), set once
        nc.gpsimd.memset(
            v_sb.rearrange("p (n c) -> p n c", c=65)[:, :, 64:65], 1.0)

        # PSUM: proj 2x[128,512]f32 (2 banks) + st 2x[128,1024]f32 (4 banks,
        # shared by scores / out-proj / recip-broadcast) + av 1x[128,1024]f32
        # (2 banks) = 8 banks.
        ps = ctx.enter_context(tc.tile_pool(name="ps", bufs=1, space="PSUM"))
        hpool = ctx.enter_context(tc.tile_pool(name="hpool", bufs=10))
        ptpool = ctx.enter_context(tc.tile_pool(name="ptpool", bufs=3))
        rcpool = ctx.enter_context(tc.tile_pool(name="rcpool", bufs=2))
        otbpool = ctx.enter_context(tc.tile_pool(name="otbpool", bufs=2))
        obpool = ctx.enter_context(tc.tile_pool(name="obpool", bufs=4))

        ht_tiles = {}

        def dma_a(bi, split=1, between=()):
            hts = [hpool.tile([128, S], dt.bfloat16, name=f"ht_{bi}_{k}",
                              tag="ht", bufs=12) for k in range(NK)]
            ht_tiles[bi] = hts
            # quarter-major issue order: the first projection chain needs
            # chunk h=0 of ALL k-tiles, so land those 8 transfers first;
            # non-critical constant loads are interleaved between quarters
            for h in range(split):
                for k in range(NK):
                    nc.sync.dma_start(
                        hts[k][:, h * (S // split):(h + 1) * (S // split)],
                        ht_d[k * 128:(k + 1) * 128,
                             bi * S + h * (S // split):
                             bi * S + (h + 1) * (S // split)])
                if h < len(between):
                    between[h]()

        def phase_a_group(bi, sub):
            """QKV projection + V transposes for one 512-token sub-block;
            everything phase_b_j(bi, j=sub) needs beyond earlier subs."""
            hts = ht_tiles[bi]
            cols = slice(bi * S + sub * SQ, bi * S + (sub + 1) * SQ)
            for w_sb, b_sb, o_sb in ((wq_sb, bq_sb, qt_sb),
                                     (wk_sb, bk_sb, kt_sb),
                                     (wv_sb, bvb_sb, vt_sb)):
                pj_ps = ps.tile([128, SQ], dt.float32, tag="proj", bufs=2,
                                name="pj_ps")
                for k in range(NK):
                    nc.tensor.matmul(
                        pj_ps[:, :], w_sb[:, k * 128:(k + 1) * 128],
                        hts[k][:, sub * SQ:(sub + 1) * SQ],
                        start=(k == 0), stop=(k == NK - 1))
                nc.vector.tensor_scalar_add(o_sb[:, cols], pj_ps[:, :],
                                            b_sb[:, 0:1])
            # transpose this sub's V^T back to V [s, hd] via the PE
            for st in range(sub * 4, sub * 4 + 4):
                g = bi * (S // 128) + st
                vtp = ps.tile([128, 128], dt.bfloat16, tag="proj", bufs=2)
                nc.tensor.transpose(
                    vtp[:, :], vt_sb[:, g * 128:(g + 1) * 128], id_sb[:, :])
                # strided copy into [vA | . | vB | .] (ones cols preserved)
                nc.vector.tensor_copy(
                    v_sb[:, g * VW:(g + 1) * VW].rearrange(
                        "p (a c) -> p a c", c=65)[:, :, 0:64],
                    vtp.rearrange("p (a c) -> p a c", a=2))
            if sub == NJ - 1:
                ht_tiles.pop(bi)

        def phase_b_j(bi, j, cbs):
            """Attention for the 2 local heads of batch bi, q-block j.
            cbs maps a kk index -> callback; each is invoked at that point
            of the emission stream. Used to place the previous q-block's
            deferred normalization tail (so its PE broadcast doesn't stall
            on ACT's ln/exp chain), the projection filler groups, and the
            deferred out-proj units (so their heavyweight DVE casts queue
            BEHIND this block's early mask-muls, which feed the PE)."""
            qcols = slice(bi * S + j * SQ, bi * S + (j + 1) * SQ)
            nk = 4 * j + 4
            av_ps = ps.tile([128, 2 * SQ], dt.float32, tag="av", bufs=1)
            # software-pipeline with a lag so the AV matmuls' exp
            # dependencies are already satisfied when the PE reaches
            # them (keeps the PE stream back-to-back)
            LAG = 5
            pts = {}

            def k_delta(k):
                # columns < delta of this kv-tile's q-range are fully
                # masked (strictly above the causal diagonal): skip them
                # in the score/AV matmuls entirely -- masked keys
                # contribute exactly zero, so this is exact.
                return (k - (nk - 4)) * 128 if k >= nk - 4 else 0

            for kk in range(nk + LAG):
                for cb in cbs.get(kk, ()):
                    cb()
                if kk < nk:
                    k = kk
                    dl = k_delta(k)
                    kvc = slice(bi * S + k * 128, bi * S + (k + 1) * 128)
                    st_ps = ps.tile([128, 2 * SQ], dt.float32, tag="st",
                                    bufs=2)
                    # scores^T for both heads, row-packed (K=64 each)
                    nc.tensor.matmul(st_ps[:, dl:SQ], kt_sb[0:64, kvc],
                                     qt_sb[0:64, qcols.start + dl:
                                           qcols.stop],
                                     start=True, stop=True)
                    nc.tensor.matmul(st_ps[:, SQ + dl:2 * SQ],
                                     kt_sb[64:128, kvc],
                                     qt_sb[64:128, qcols.start + dl:
                                           qcols.stop],
                                     start=True, stop=True)
                    pt = ptpool.tile([128, 2 * SQ], dt.bfloat16,
                                     tag="pt", bufs=7)
                    pts[k] = pt
                    if dl > 0:
                        # one 2D-AP exp covering both heads' live columns
                        nc.scalar.activation(
                            pt.rearrange("p (h c) -> p h c", h=2)[:, :, dl:SQ],
                            st_ps.rearrange("p (h c) -> p h c", h=2)[:, :, dl:SQ],
                            AF.Exp, scale=SCALE)
                    else:
                        nc.scalar.activation(pt[:, :], st_ps[:, :],
                                             AF.Exp, scale=SCALE)
                    if k >= nk - 4:
                        # mask only the live columns (2D AP, both heads)
                        midx = k - (nk - 4)
                        mrow = mask_sb[:, midx * 1024:(midx + 1) * 1024]
                        nc.vector.tensor_mul(
                            pt.rearrange("p (h c) -> p h c", h=2)[:, :, dl:SQ],
                            pt.rearrange("p (h c) -> p h c", h=2)[:, :, dl:SQ],
                            mrow.rearrange("p (h c) -> p h c", h=2)[:, :, dl:SQ])
                    if taps is not None and bi == 0 and j == 0 and k == 0:
                        nc.sync.dma_start(taps["dpt"][:, :], pt[:, :])
                if kk >= LAG:
                    k = kk - LAG
                    dl = k_delta(k)
                    pt = pts.pop(k)
                    # AV for both heads into rows 0:65 of one PSUM tile:
                    # head A -> cols 0:512, head B -> cols 512:1024. The
                    # 65th stationary column is ones, so row 64 of each
                    # half accumulates the softmax denominator for free.
                    g = bi * (S // 128) + k
                    va = v_sb[:, g * VW:g * VW + 65]
                    vb = v_sb[:, g * VW + 65:g * VW + VW]
                    first, last = (k == 0), (k == nk - 1)
                    nc.tensor.matmul(av_ps[0:65, dl:SQ], va,
                                     pt[:, dl:SQ],
                                     start=first, stop=last)
                    nc.tensor.matmul(av_ps[0:65, SQ + dl:2 * SQ], vb,
                                     pt[:, SQ + dl:2 * SQ],
                                     start=first, stop=last,
                                     skip_group_check=True)

            if taps is not None and bi == 0 and j == 0:
                davs = rcpool.tile([128, 2 * SQ], dt.float32, tag="dav", bufs=1)
                nc.scalar.copy(davs[:, :], av_ps[:, :])
                nc.sync.dma_start(taps["dav"][:, :], davs[:, :])

            # ---- softmax normalization, part 1 (emitted now, runs on ACT
            # while the next q-block's scores keep the PE busy):
            # 1/den = exp(-ln(den)); both heads' dens sit on partition 64
            # (cols 0:512 = head A, 512:1024 = head B) ----
            sc = rcpool.tile([128, 2 * SQ], dt.float32, tag="sc", bufs=2)
            rc = rcpool.tile([128, 2 * SQ], dt.bfloat16, tag="rc", bufs=2)
            nc.scalar.activation(sc[64:65, :], av_ps[64:65, :], AF.Ln)
            nc.scalar.activation(rc[64:65, :], sc[64:65, :],
                                 AF.Exp, scale=-1.0)

            def norm_tail():
                # part 2: broadcast 1/den across the 64 head partitions via
                # two PE matmuls (ones lhsT at partition 64), one PSUM->SBUF
                # copy, then the normalizing multiplies.
                bc_ps = ps.tile([128, 2 * SQ], dt.float32, tag="st", bufs=2)
                nc.tensor.matmul(bc_ps[0:64, 0:SQ], ones_row[64:65, :],
                                 rc[64:65, 0:SQ], start=True, stop=True)
                nc.tensor.matmul(bc_ps[0:64, SQ:2 * SQ], ones_row[64:65, :],
                                 rc[64:65, SQ:2 * SQ], start=True, stop=True,
                                 skip_group_check=True)
                bc_sb = rcpool.tile([128, 2 * SQ], dt.float32, tag="bc",
                                    bufs=2)
                nc.vector.tensor_copy(bc_sb[0:64, :], bc_ps[0:64, :])
                nc.vector.tensor_mul(ot_sb[0:64, qcols], av_ps[0:64, 0:SQ],
                                     bc_sb[0:64, 0:SQ])
                otb = otbpool.tile([64, SQ], dt.bfloat16, tag="otb", bufs=2)
                nc.vector.tensor_mul(otb[0:64, :],
                                     av_ps[0:64, SQ:2 * SQ],
                                     bc_sb[0:64, SQ:2 * SQ])
                # head B lives at ot rows 64:128; DVE can't shift
                # partitions, so hop through a small SBUF->SBUF DMA
                nc.sync.dma_start(ot_sb[64:128, qcols], otb[0:64, :])
                if taps is not None and bi == 0 and j == 0:
                    nc.sync.dma_start(taps["drc"][:, :], rc[:, :])
                    nc.sync.dma_start(taps["dbc"][:, :], bc_sb[:, :])

            return norm_tail

        def phase_c_sp(bi, sp, act_ok=False):
            # output projection for one pair of s-tiles of batch bi; PSUM
            # can't be DMA'd directly, so hop through a bf16 SBUF staging
            # tile (conversion halves the HBM write)
            for half in range(2):
                t = bi * (S // 128) + sp * 2 + half
                op_ps = ps.tile([128, 1024], dt.float32, tag="st", bufs=2)
                lhs = ot_sb[:, t * 128:(t + 1) * 128]
                nc.tensor.matmul(op_ps[:, 0:512], lhs, wo_sb[:, 0:512],
                                 start=True, stop=True)
                nc.tensor.matmul(op_ps[:, 512:1024], lhs, wo_sb[:, 512:1024],
                                 start=True, stop=True)
                ob = obpool.tile([128, 1024], dt.bfloat16, tag="ob", bufs=4)
                if half == 1 and act_ok:
                    # split the PSUM drains across DVE and the scalar
                    # engine, but only inside long kv-blocks where ACT
                    # has per-tile slack over the PE
                    nc.scalar.copy(ob[:, :], op_ps[:, :])
                else:
                    nc.vector.tensor_copy(ob[:, :], op_ps[:, :])
                row0 = bi * S + sp * 256 + half * 128
                nc.sync.dma_start(out_d[row0:row0 + 128, :], ob[:, :])

        # Emission plan: the per-engine instruction streams are fixed at
        # compile time, so PE-dense filler must be emitted inside the
        # attention stream where the exp-dependency stalls happen. Each
        # q-block of attention gets exactly one projection sub-group,
        # emitted two q-blocks ahead of its consumer -- uniform filler
        # density across the whole kernel -- plus deferred out-proj units.
        groups = [(bi, sub) for bi in range(B) for sub in range(NJ)]
        # startup: the first projection chain needs wq + bq + chunk 0 of
        # batch 0's hidden states; everything else lands between quarters
        load_w(wq_d, wq_sb)
        nc.sync.dma_start(bq_sb[:, :], bq_d[:, :])
        nc.sync.dma_start(id_sb[:, :], id_d[:, :])
        dma_a(0, split=4, between=(
            lambda: (load_w(wk_d, wk_sb),
                     nc.sync.dma_start(bk_sb[:, :], bk_d[:, :]),
                     load_w(wv_d, wv_sb),
                     nc.sync.dma_start(bvb_sb[:, :], bvb_d[:, :])),
            lambda: nc.sync.dma_start(mask_sb[:, :], mk_d[:, :]),
            lambda: nc.sync.dma_start(wo_sb[:, :], wo_d[:, :]),
        ))
        phase_a_group(0, 0)
        phase_a_group(0, 1)
        gidx = 2
        deferred_c = collections.deque()
        pending_norm = None
        for bi in range(B):
            for j in range(NJ):
                gj = 4 * bi + j
                cbs = collections.defaultdict(list)
                if j == 0 and bi + 1 < B:
                    cbs[0].append(lambda bn=bi + 1: dma_a(bn))
                while gidx < len(groups) and \
                        groups[gidx][0] * 4 + groups[gidx][1] <= gj + 2:
                    cbs[1].append(
                        lambda g=groups[gidx]: phase_a_group(*g))
                    gidx += 1
                if pending_norm is not None:
                    cbs[4].append(pending_norm)
                npop = 2 if gj >= 2 else 0
                if gj >= 13:
                    npop = 3
                for _ in range(min(npop, len(deferred_c))):
                    cbs[min(5, 4 * j + 4)].append(
                        lambda u=deferred_c.popleft(), a=(j >= 2):
                            phase_c_sp(*u, act_ok=a))
                pending_norm = phase_b_j(bi, j, cbs)
                deferred_c.append((bi, 2 * j))
                deferred_c.append((bi, 2 * j + 1))
        while gidx < len(groups):
            phase_a_group(*groups[gidx])
            gidx += 1
        if pending_norm is not None:
            pending_norm()
        while deferred_c:
            phase_c_sp(*deferred_c.popleft())

        if taps is not None:
            nc.sync.dma_start(taps["dqt"][:, :], qt_sb[:, :])
            nc.sync.dma_start(taps["dkt"][:, :], kt_sb[:, :])
            nc.sync.dma_start(taps["dv"][:, :], v_sb[:, :])
            nc.sync.dma_start(taps["dot"][:, :], ot_sb[:, :])


def _get_nc():
    if "nc" not in _CACHE:
        _CACHE["nc"] = _build_nc()
    return _CACHE["nc"]


def _build_mask():
    # mask[kv, q] for the 4 diagonal sub-tiles: delta = 0, 128, 256, 384.
    # allowed iff kv_local <= q_local - delta. Each [128, 512] block is
    # duplicated for the two heads -> [128, 1024] per delta, 4 deltas.
    i = np.arange(128)[:, None]
    q = np.arange(SQ)[None, :]
    blocks = []
    for delta in (0, 128, 256, 384):
        m = (i <= (q - delta)).astype(np.float32)
        blocks.append(np.concatenate([m, m], axis=1))
    return np.concatenate(blocks, axis=1).astype(BF16)


def kernel(hidden_states, Wq, bq, Wk, bk, Wv, bv, Wo, bo):
    global LAST_RESULTS
    from concourse import bass_utils

    nc = _get_nc()

    hid = np.ascontiguousarray(
        np.asarray(hidden_states, dtype=np.float32).reshape(BS, D).T)
    ht = hid.astype(BF16)
    mask = _build_mask()
    Wq = np.asarray(Wq, np.float32)
    Wk = np.asarray(Wk, np.float32)
    Wv = np.asarray(Wv, np.float32)
    Wo = np.asarray(Wo, np.float32)
    bq = np.asarray(bq, np.float32)
    bk = np.asarray(bk, np.float32)
    bv = np.asarray(bv, np.float32)
    bo = np.asarray(bo, np.float32)

    in_maps = []
    for c in range(NCORES):
        sl = slice(DL * c, DL * (c + 1))
        in_maps.append({
            "ht": ht,
            "wq": np.ascontiguousarray(Wq[:, sl]).astype(BF16),
            "wk": np.ascontiguousarray(Wk[:, sl]).astype(BF16),
            "wv": np.ascontiguousarray(Wv[:, sl]).astype(BF16),
            "wo": np.ascontiguousarray(Wo[sl, :]).astype(BF16),
            "bq": np.ascontiguousarray(bq[sl]).reshape(DL, 1),
            "bk": np.ascontiguousarray(bk[sl]).reshape(DL, 1),
            "bvb": np.ascontiguousarray(bv[sl]).reshape(DL, 1),
            "ident": np.eye(128, dtype=np.float32).astype(BF16),
            "mask": mask,
        })

    res = bass_utils.run_bass_kernel_spmd(
        nc, in_maps, core_ids=list(range(NCORES)))
    LAST_RESULTS = res

    out = res.results[0]["out"].astype(np.float32).copy()
    for c in range(1, NCORES):
        out += res.results[c]["out"]
    out += bo[None, :]
    return out.reshape(B, S, D)


# revision 38
# speedup vs baseline: 1.0673x; 1.0673x over previous
"""Trainium2 Bass kernel for CustomGPT2Attention (B=4, S=2048, D=1024, H=16).

Strategy: tensor-parallel over heads. Each of the 8 NeuronCores owns 2 heads
(a 128-wide slice of the QKV projections and the matching 128 rows of Wo),
computes its partial output projection over the full batch, and the host sums
the 8 partials (the "all-reduce" of the row-parallel c_proj) plus bo.

All on-device layouts are chosen so no transposes are ever needed:
  - host ships hidden^T [D, B*S] (bf16)
  - QT, KT come out of the projection as [hd_local, B*S]
  - scores are computed transposed, ST[kv, q] = K^T Q, softmax runs along
    the partition (kv) axis; the denominators come for free from a 65th
    ones-column appended to each AV stationary (V tile), so no separate
    denominator matmuls are needed
  - AV output OT[hd_local, q] is directly the lhsT of the output projection
  - the output projection result is DMA'd straight from PSUM to DRAM
Compute dtype bf16 (fp32 PSUM accumulation everywhere).
"""

import collections
import os
import sys

for _p in ("/opt/trn_rl_repo", "/root/.axon_site/_ro/trn_rl_repo"):
    if os.path.isdir(_p) and _p not in sys.path:
        sys.path.insert(0, _p)

import numpy as np
import ml_dtypes

BF16 = ml_dtypes.bfloat16

B, S, D, H, HD = 4, 2048, 1024, 16, 64
BS = B * S            # 8192 tokens
NCORES = 8
DL = D // NCORES      # 128 = per-core slice (2 heads x 64)
NK = D // 128         # 8 contraction chunks for the projections
SQ = 512              # q free-block width
NJ = S // SQ          # 4 q-blocks per batch
NT = BS // 128        # 64 s-tiles (of 128 tokens)
SCALE = 1.0 / 8.0     # 1/sqrt(HD)
VW = 130              # v_sb per-tile width: [vA(64) | 1 | vB(64) | 1]

_CACHE = {}
LAST_RESULTS = None
KDEBUG = bool(os.environ.get("KDEBUG"))


def _build_nc():
    import concourse.bacc as bacc
    import concourse.tile as tile
    import concourse.mybir as mybir
    import bass_rust

    dt = mybir.dt
    AF = mybir.ActivationFunctionType

    class _Bacc(bacc.Bacc):
        # All ACT functions we use (Exp, Ln) live in the
        # natural_log_exp_and_others table set. The stock table-load pass
        # assigns each function its first matching set, which thrashes
        # ACT_TABLE_LOADs (~1.3us each) between exp/ln sets inside the
        # softmax loop. Restrict the pass to the one set that has them all.
        def insert_act_table_loads(self):
            from concourse.hw_specs import get_activation_tables
            has_activation = any(
                isinstance(i, mybir.InstActivation)
                for b in self.main_func.blocks
                for i in b.instructions
            )
            if not has_activation:
                return
            tables = []
            for name, funcs in get_activation_tables(self.m.arch).items():
                if name != "natural_log_exp_and_others":
                    funcs = set()
                tables.append((name, funcs))
            bass_rust.insert_act_table_loads(self, tables)

    nc = _Bacc(
        "TRN2", target_bir_lowering=False, debug=False, num_devices=NCORES
    )

    ht_d = nc.dram_tensor("ht", [D, BS], dt.bfloat16, kind="ExternalInput").ap()
    wq_d = nc.dram_tensor("wq", [D, DL], dt.bfloat16, kind="ExternalInput").ap()
    wk_d = nc.dram_tensor("wk", [D, DL], dt.bfloat16, kind="ExternalInput").ap()
    wv_d = nc.dram_tensor("wv", [D, DL], dt.bfloat16, kind="ExternalInput").ap()
    wo_d = nc.dram_tensor("wo", [DL, D], dt.bfloat16, kind="ExternalInput").ap()
    bq_d = nc.dram_tensor("bq", [DL, 1], dt.float32, kind="ExternalInput").ap()
    bk_d = nc.dram_tensor("bk", [DL, 1], dt.float32, kind="ExternalInput").ap()
    bvb_d = nc.dram_tensor("bvb", [DL, 1], dt.float32, kind="ExternalInput").ap()
    id_d = nc.dram_tensor("ident", [128, 128], dt.bfloat16, kind="ExternalInput").ap()
    mk_d = nc.dram_tensor("mask", [128, 4 * 1024], dt.bfloat16, kind="ExternalInput").ap()
    out_d = nc.dram_tensor("out", [BS, D], dt.bfloat16, kind="ExternalOutput").ap()
    otd_d = nc.dram_tensor("otshift", [64, BS], dt.bfloat16, kind="Internal").ap()

    taps = None
    if KDEBUG:
        taps = {
            "dqt": nc.dram_tensor("dqt", [128, BS], dt.bfloat16, kind="ExternalOutput").ap(),
            "dkt": nc.dram_tensor("dkt", [128, BS], dt.bfloat16, kind="ExternalOutput").ap(),
            "dv": nc.dram_tensor("dv", [128, VW * NT], dt.bfloat16, kind="ExternalOutput").ap(),
            "dot": nc.dram_tensor("dot", [128, BS], dt.bfloat16, kind="ExternalOutput").ap(),
            "dav": nc.dram_tensor("dav", [128, 1024], dt.float32, kind="ExternalOutput").ap(),
            "drc": nc.dram_tensor("drc", [128, 1024], dt.bfloat16, kind="ExternalOutput").ap(),
            "dbc": nc.dram_tensor("dbc", [128, 1024], dt.float32, kind="ExternalOutput").ap(),
            "dpt": nc.dram_tensor("dpt", [128, 1024], dt.bfloat16, kind="ExternalOutput").ap(),
        }

    with tile.TileContext(nc) as tc:
        _body(tc, nc, mybir, ht_d, wq_d, wk_d, wv_d, wo_d, bq_d, bk_d, bvb_d,
              id_d, mk_d, out_d, otd_d, taps)

    nc.compile()
    return nc


def _body(tc, nc, mybir, ht_d, wq_d, wk_d, wv_d, wo_d, bq_d, bk_d, bvb_d,
          id_d, mk_d, out_d, otd_d, taps=None):
    from contextlib import ExitStack

    dt = mybir.dt
    AF = mybir.ActivationFunctionType

    ctx = ExitStack()
    with ctx:
        consts = ctx.enter_context(tc.tile_pool(name="consts", bufs=1))

        # --- constants / weights (persist whole kernel) ---
        wq_sb = consts.tile([128, D], dt.bfloat16, name="wq_sb")
        wk_sb = consts.tile([128, D], dt.bfloat16, name="wk_sb")
        wv_sb = consts.tile([128, D], dt.bfloat16, name="wv_sb")
        wo_sb = consts.tile([128, D], dt.bfloat16, name="wo_sb")
        # Wo's bottom 64 rows re-homed at partitions 0:64, so the final
        # q-block's out-proj can read head B's attn output straight from
        # the otb staging tile (split-K) instead of waiting on a DMA shift
        wo2_sb = consts.tile([64, D], dt.bfloat16, name="wo2_sb")
        bq_sb = consts.tile([128, 1], dt.float32, name="bq_sb")
        bk_sb = consts.tile([128, 1], dt.float32, name="bk_sb")
        bvb_sb = consts.tile([128, 1], dt.float32, name="bvb_sb")
        id_sb = consts.tile([128, 128], dt.bfloat16, name="id_sb")
        mask_sb = consts.tile([128, 4 * 1024], dt.bfloat16, name="mask_sb")
        ones_row = consts.tile([128, 64], dt.bfloat16, name="ones_row")

        # weights are [D, DL] in DRAM; load as 8 lhsT tiles [128, 128] side by
        # side -> SBUF [128, 8*128]
        def load_w(w_d, w_sb):
            nc.sync.dma_start(
                w_sb.rearrange("p (k n) -> p k n", k=NK),
                w_d.rearrange("(k p) n -> p k n", p=128),
            )
        nc.gpsimd.memset(ones_row[:, :], 1.0)

        # --- persistent activation tensors ---
        qt_sb = consts.tile([128, BS], dt.bfloat16, name="qt_sb")   # Q^T
        kt_sb = consts.tile([128, BS], dt.bfloat16, name="kt_sb")   # K^T
        vt_sb = consts.tile([128, BS], dt.bfloat16, name="vt_sb")   # V^T
        # V with a ones column per head: s-tile t occupies cols
        # [t*VW, (t+1)*VW) as [vA(64) | 1 | vB(64) | 1]; the AV stationaries
        # are the contiguous 65-col slices [0:65] and [65:130].
        v_sb = consts.tile([128, VW * NT], dt.bfloat16, name="v_sb")
        ot_sb = consts.tile([128, BS], dt.bfloat16, name="ot_sb")   # attn out ^T

        # ones columns (col 64 of each 65-wide half-tile), set once
        nc.gpsimd.memset(
            v_sb.rearrange("p (n c) -> p n c", c=65)[:, :, 64:65], 1.0)

        # PSUM: proj 2x[128,512]f32 (2 banks) + st 2x[128,1024]f32 (4 banks,
        # shared by scores / out-proj / recip-broadcast) + av 1x[128,1024]f32
        # (2 banks) = 8 banks.
        ps = ctx.enter_context(tc.tile_pool(name="ps", bufs=1, space="PSUM"))
        hpool = ctx.enter_context(tc.tile_pool(name="hpool", bufs=10))
        ptpool = ctx.enter_context(tc.tile_pool(name="ptpool", bufs=3))
        rcpool = ctx.enter_context(tc.tile_pool(name="rcpool", bufs=2))
        otbpool = ctx.enter_context(tc.tile_pool(name="otbpool", bufs=2))
        obpool = ctx.enter_context(tc.tile_pool(name="obpool", bufs=4))

        ht_tiles = {}

        def dma_a(bi, split=1, between=()):
            hts = [hpool.tile([128, S], dt.bfloat16, name=f"ht_{bi}_{k}",
                              tag="ht", bufs=12) for k in range(NK)]
            ht_tiles[bi] = hts
            # quarter-major issue order: the first projection chain needs
            # chunk h=0 of ALL k-tiles, so land those 8 transfers first;
            # non-critical constant loads are interleaved between quarters
            for h in range(split):
                for k in range(NK):
                    nc.sync.dma_start(
                        hts[k][:, h * (S // split):(h + 1) * (S // split)],
                        ht_d[k * 128:(k + 1) * 128,
                             bi * S + h * (S // split):
                             bi * S + (h + 1) * (S // split)])
                if h < len(between):
                    between[h]()

        def phase_a_group(bi, sub):
            """QKV projection + V transposes for one 512-token sub-block;
            everything phase_b_j(bi, j=sub) needs beyond earlier subs."""
            hts = ht_tiles[bi]
            cols = slice(bi * S + sub * SQ, bi * S + (sub + 1) * SQ)
            for w_sb, b_sb, o_sb in ((wq_sb, bq_sb, qt_sb),
                                     (wk_sb, bk_sb, kt_sb),
                                     (wv_sb, bvb_sb, vt_sb)):
                pj_ps = ps.tile([128, SQ], dt.float32, tag="proj", bufs=2,
                                name="pj_ps")
                for k in range(NK):
                    nc.tensor.matmul(
                        pj_ps[:, :], w_sb[:, k * 128:(k + 1) * 128],
                        hts[k][:, sub * SQ:(sub + 1) * SQ],
                        start=(k == 0), stop=(k == NK - 1))
                nc.vector.tensor_scalar_add(o_sb[:, cols], pj_ps[:, :],
                                            b_sb[:, 0:1])
            # transpose this sub's V^T back to V [s, hd] via the PE
            for st in range(sub * 4, sub * 4 + 4):
                g = bi * (S // 128) + st
                vtp = ps.tile([128, 128], dt.bfloat16, tag="proj", bufs=2)
                nc.tensor.transpose(
                    vtp[:, :], vt_sb[:, g * 128:(g + 1) * 128], id_sb[:, :])
                # strided copy into [vA | . | vB | .] (ones cols preserved)
                nc.vector.tensor_copy(
                    v_sb[:, g * VW:(g + 1) * VW].rearrange(
                        "p (a c) -> p a c", c=65)[:, :, 0:64],
                    vtp.rearrange("p (a c) -> p a c", a=2))
            if sub == NJ - 1:
                ht_tiles.pop(bi)

        last_otb = [None]

        def phase_b_j(bi, j, cbs, final=False):
            """Attention for the 2 local heads of batch bi, q-block j.
            cbs maps a kk index -> callback; each is invoked at that point
            of the emission stream. Used to place the previous q-block's
            deferred normalization tail (so its PE broadcast doesn't stall
            on ACT's ln/exp chain), the projection filler groups, and the
            deferred out-proj units (so their heavyweight DVE casts queue
            BEHIND this block's early mask-muls, which feed the PE)."""
            qcols = slice(bi * S + j * SQ, bi * S + (j + 1) * SQ)
            nk = 4 * j + 4
            av_ps = ps.tile([128, 2 * SQ], dt.float32, tag="av", bufs=1)
            # software-pipeline with a lag so the AV matmuls' exp
            # dependencies are already satisfied when the PE reaches
            # them (keeps the PE stream back-to-back)
            LAG = 4
            pts = {}

            def k_delta(k):
                # columns < delta of this kv-tile's q-range are fully
                # masked (strictly above the causal diagonal): skip them
                # in the score/AV matmuls entirely -- masked keys
                # contribute exactly zero, so this is exact.
                return (k - (nk - 4)) * 128 if k >= nk - 4 else 0

            for kk in range(nk + LAG):
                for cb in cbs.get(kk, ()):
                    cb()
                if kk < nk:
                    k = kk
                    dl = k_delta(k)
                    kvc = slice(bi * S + k * 128, bi * S + (k + 1) * 128)
                    st_ps = ps.tile([128, 2 * SQ], dt.float32, tag="st",
                                    bufs=2)
                    # scores^T for both heads, row-packed (K=64 each)
                    nc.tensor.matmul(st_ps[:, dl:SQ], kt_sb[0:64, kvc],
                                     qt_sb[0:64, qcols.start + dl:
                                           qcols.stop],
                                     start=True, stop=True)
                    nc.tensor.matmul(st_ps[:, SQ + dl:2 * SQ],
                                     kt_sb[64:128, kvc],
                                     qt_sb[64:128, qcols.start + dl:
                                           qcols.stop],
                                     start=True, stop=True)
                    pt = ptpool.tile([128, 2 * SQ], dt.bfloat16,
                                     tag="pt", bufs=6)
                    pts[k] = pt
                    if dl > 0:
                        # one 2D-AP exp covering both heads' live columns
                        nc.scalar.activation(
                            pt.rearrange("p (h c) -> p h c", h=2)[:, :, dl:SQ],
                            st_ps.rearrange("p (h c) -> p h c", h=2)[:, :, dl:SQ],
                            AF.Exp, scale=SCALE)
                    else:
                        nc.scalar.activation(pt[:, :], st_ps[:, :],
                                             AF.Exp, scale=SCALE)
                    if k >= nk - 4:
                        # mask only the live columns (2D AP, both heads)
                        midx = k - (nk - 4)
                        mrow = mask_sb[:, midx * 1024:(midx + 1) * 1024]
                        nc.vector.tensor_mul(
                            pt.rearrange("p (h c) -> p h c", h=2)[:, :, dl:SQ],
                            pt.rearrange("p (h c) -> p h c", h=2)[:, :, dl:SQ],
                            mrow.rearrange("p (h c) -> p h c", h=2)[:, :, dl:SQ])
                    if taps is not None and bi == 0 and j == 0 and k == 0:
                        nc.sync.dma_start(taps["dpt"][:, :], pt[:, :])
                if kk >= LAG:
                    k = kk - LAG
                    dl = k_delta(k)
                    pt = pts.pop(k)
                    # AV for both heads into rows 0:65 of one PSUM tile:
                    # head A -> cols 0:512, head B -> cols 512:1024. The
                    # 65th stationary column is ones, so row 64 of each
                    # half accumulates the softmax denominator for free.
                    g = bi * (S // 128) + k
                    va = v_sb[:, g * VW:g * VW + 65]
                    vb = v_sb[:, g * VW + 65:g * VW + VW]
                    first, last = (k == 0), (k == nk - 1)
                    nc.tensor.matmul(av_ps[0:65, dl:SQ], va,
                                     pt[:, dl:SQ],
                                     start=first, stop=last)
                    nc.tensor.matmul(av_ps[0:65, SQ + dl:2 * SQ], vb,
                                     pt[:, SQ + dl:2 * SQ],
                                     start=first, stop=last,
                                     skip_group_check=True)

            if taps is not None and bi == 0 and j == 0:
                davs = rcpool.tile([128, 2 * SQ], dt.float32, tag="dav", bufs=1)
                nc.scalar.copy(davs[:, :], av_ps[:, :])
                nc.sync.dma_start(taps["dav"][:, :], davs[:, :])

            # ---- softmax normalization, part 1 (emitted now, runs on ACT
            # while the next q-block's scores keep the PE busy):
            # 1/den = exp(-ln(den)); both heads' dens sit on partition 64
            # (cols 0:512 = head A, 512:1024 = head B) ----
            sc = rcpool.tile([128, 2 * SQ], dt.float32, tag="sc", bufs=2)
            rc = rcpool.tile([128, 2 * SQ], dt.bfloat16, tag="rc", bufs=2)
            nc.scalar.activation(sc[64:65, :], av_ps[64:65, :], AF.Ln)
            nc.scalar.activation(rc[64:65, :], sc[64:65, :],
                                 AF.Exp, scale=-1.0)

            def norm_tail():
                # part 2: broadcast 1/den across the 64 head partitions via
                # two PE matmuls (ones lhsT at partition 64), one PSUM->SBUF
                # copy, then the normalizing multiplies.
                bc_ps = ps.tile([128, 2 * SQ], dt.float32, tag="st", bufs=2)
                nc.tensor.matmul(bc_ps[0:64, 0:SQ], ones_row[64:65, :],
                                 rc[64:65, 0:SQ], start=True, stop=True)
                nc.tensor.matmul(bc_ps[0:64, SQ:2 * SQ], ones_row[64:65, :],
                                 rc[64:65, SQ:2 * SQ], start=True, stop=True,
                                 skip_group_check=True)
                bc_sb = rcpool.tile([128, 2 * SQ], dt.float32, tag="bc",
                                    bufs=2)
                nc.vector.tensor_copy(bc_sb[0:64, :], bc_ps[0:64, :])
                nc.vector.tensor_mul(ot_sb[0:64, qcols], av_ps[0:64, 0:SQ],
                                     bc_sb[0:64, 0:SQ])
                otb = otbpool.tile([64, SQ], dt.bfloat16, tag="otb", bufs=2)
                nc.vector.tensor_mul(otb[0:64, :],
                                     av_ps[0:64, SQ:2 * SQ],
                                     bc_sb[0:64, SQ:2 * SQ])
                # head B lives at ot rows 64:128; DVE can't shift
                # partitions, so hop through a small SBUF->SBUF DMA. The
                # final q-block skips the shift: its out-proj units read
                # otb directly via split-K (shorter tail, no DMA wait).
                if not final:
                    nc.sync.dma_start(ot_sb[64:128, qcols], otb[0:64, :])
                else:
                    last_otb[0] = otb
                if taps is not None and bi == 0 and j == 0:
                    nc.sync.dma_start(taps["drc"][:, :], rc[:, :])
                    nc.sync.dma_start(taps["dbc"][:, :], bc_sb[:, :])

            return norm_tail

        def phase_c_sp(bi, sp, act_ok=False, final=False):
            # output projection for one pair of s-tiles of batch bi; PSUM
            # can't be DMA'd directly, so hop through a bf16 SBUF staging
            # tile (conversion halves the HBM write)
            for half in range(2):
                t = bi * (S // 128) + sp * 2 + half
                op_ps = ps.tile([128, 1024], dt.float32, tag="st", bufs=2)
                lhs = ot_sb[:, t * 128:(t + 1) * 128]
                if final:
                    # split-K: head A from ot_sb rows 0:64, head B straight
                    # from the otb staging tile (its DRAM shift is skipped)
                    otb = last_otb[0]
                    oc = (t % 4) * 128
                    for c0 in (0, 512):
                        nc.tensor.matmul(
                            op_ps[:, c0:c0 + 512], lhs[0:64, :],
                            wo_sb[0:64, c0:c0 + 512], start=True, stop=False)
                        nc.tensor.matmul(
                            op_ps[:, c0:c0 + 512], otb[0:64, oc:oc + 128],
                            wo2_sb[0:64, c0:c0 + 512], start=False, stop=True)
                else:
                    nc.tensor.matmul(op_ps[:, 0:512], lhs, wo_sb[:, 0:512],
                                     start=True, stop=True)
                    nc.tensor.matmul(op_ps[:, 512:1024], lhs,
                                     wo_sb[:, 512:1024],
                                     start=True, stop=True)
                ob = obpool.tile([128, 1024], dt.bfloat16, tag="ob", bufs=4)
                if half == 1 and act_ok:
                    # split the PSUM drains across DVE and the scalar
                    # engine, but only inside long kv-blocks where ACT
                    # has per-tile slack over the PE
                    nc.scalar.copy(ob[:, :], op_ps[:, :])
                else:
                    nc.vector.tensor_copy(ob[:, :], op_ps[:, :])
                row0 = bi * S + sp * 256 + half * 128
                nc.sync.dma_start(out_d[row0:row0 + 128, :], ob[:, :])

        # Emission plan: the per-engine instruction streams are fixed at
        # compile time, so PE-dense filler must be emitted inside the
        # attention stream where the exp-dependency stalls happen. Each
        # q-block of attention gets exactly one projection sub-group,
        # emitted two q-blocks ahead of its consumer -- uniform filler
        # density across the whole kernel -- plus deferred out-proj units.
        groups = [(bi, sub) for bi in range(B) for sub in range(NJ)]
        # startup: the first projection chain needs wq + bq + chunk 0 of
        # batch 0's hidden states; everything else lands between quarters
        load_w(wq_d, wq_sb)
        nc.sync.dma_start(bq_sb[:, :], bq_d[:, :])
        nc.sync.dma_start(id_sb[:, :], id_d[:, :])
        dma_a(0, split=4, between=(
            lambda: (load_w(wk_d, wk_sb),
                     nc.sync.dma_start(bk_sb[:, :], bk_d[:, :]),
                     load_w(wv_d, wv_sb),
                     nc.sync.dma_start(bvb_sb[:, :], bvb_d[:, :])),
            lambda: nc.sync.dma_start(mask_sb[:, :], mk_d[:, :]),
            lambda: (nc.sync.dma_start(wo_sb[:, :], wo_d[:, :]),
                     nc.sync.dma_start(wo2_sb[:, :], wo_d[64:128, :])),
        ))
        phase_a_group(0, 0)
        phase_a_group(0, 1)
        gidx = 2
        deferred_c = collections.deque()
        pending_norm = None
        for bi in range(B):
            for j in range(NJ):
                gj = 4 * bi + j
                cbs = collections.defaultdict(list)
                if j == 0 and bi + 1 < B:
                    dma_a(bi + 1)
                while gidx < len(groups) and \
                        groups[gidx][0] * 4 + groups[gidx][1] <= gj + 2:
                    phase_a_group(*groups[gidx])
                    gidx += 1
                if pending_norm is not None:
                    cbs[min(3, 4 * j + 3)].append(pending_norm)
                # pop BEFORE appending this block's units: popping units of
                # the still-unnormalized current q-block would read stale ot
                npop = 2 if gj >= 2 else 0
                if gj >= 13:
                    npop = 3
                popped = [deferred_c.popleft()
                          for _ in range(min(npop, len(deferred_c)))]
                pending_norm = phase_b_j(bi, j, cbs, final=(gj == 15))
                deferred_c.append((bi, 2 * j))
                deferred_c.append((bi, 2 * j + 1))
                for u in popped:
                    phase_c_sp(*u)
        while gidx < len(groups):
            phase_a_group(*groups[gidx])
            gidx += 1
        if pending_norm is not None:
            pending_norm()
        while deferred_c:
            bi_, sp_ = deferred_c.popleft()
            phase_c_sp(bi_, sp_, final=(bi_ == 3 and sp_ >= 6))

        if taps is not None:
            nc.sync.dma_start(taps["dqt"][:, :], qt_sb[:, :])
            nc.sync.dma_start(taps["dkt"][:, :], kt_sb[:, :])
            nc.sync.dma_start(taps["dv"][:, :], v_sb[:, :])
            nc.sync.dma_start(taps["dot"][:, :], ot_sb[:, :])


def _get_nc():
    if "nc" not in _CACHE:
        _CACHE["nc"] = _build_nc()
    return _CACHE["nc"]


def _build_mask():
    # mask[kv, q] for the 4 diagonal sub-tiles: delta = 0, 128, 256, 384.
    # allowed iff kv_local <= q_local - delta. Each [128, 512] block is
    # duplicated for the two heads -> [128, 1024] per delta, 4 deltas.
    i = np.arange(128)[:, None]
    q = np.arange(SQ)[None, :]
    blocks = []
    for delta in (0, 128, 256, 384):
        m = (i <= (q - delta)).astype(np.float32)
        blocks.append(np.concatenate([m, m], axis=1))
    return np.concatenate(blocks, axis=1).astype(BF16)


def kernel(hidden_states, Wq, bq, Wk, bk, Wv, bv, Wo, bo):
    global LAST_RESULTS
    from concourse import bass_utils

    nc = _get_nc()

    hid = np.ascontiguousarray(
        np.asarray(hidden_states, dtype=np.float32).reshape(BS, D).T)
    ht = hid.astype(BF16)
    mask = _build_mask()
    Wq = np.asarray(Wq, np.float32)
    Wk = np.asarray(Wk, np.float32)
    Wv = np.asarray(Wv, np.float32)
    Wo = np.asarray(Wo, np.float32)
    bq = np.asarray(bq, np.float32)
    bk = np.asarray(bk, np.float32)
    bv = np.asarray(bv, np.float32)
    bo = np.asarray(bo, np.float32)

    in_maps = []
    for c in range(NCORES):
        sl = slice(DL * c, DL * (c + 1))
        in_maps.append({
            "ht": ht,
            "wq": np.ascontiguousarray(Wq[:, sl]).astype(BF16),
            "wk": np.ascontiguousarray(Wk[:, sl]).astype(BF16),
            "wv": np.ascontiguousarray(Wv[:, sl]).astype(BF16),
            "wo": np.ascontiguousarray(Wo[sl, :]).astype(BF16),
            "bq": np.ascontiguousarray(bq[sl]).reshape(DL, 1),
            "bk": np.ascontiguousarray(bk[sl]).reshape(DL, 1),
            "bvb": np.ascontiguousarray(bv[sl]).reshape(DL, 1),
            "ident": np.eye(128, dtype=np.float32).astype(BF16),
            "mask": mask,
        })

    res = bass_utils.run_bass_kernel_spmd(
        nc, in_maps, core_ids=list(range(NCORES)))
    LAST_RESULTS = res

    out = res.results[0]["out"].astype(np.float32).copy()
    for c in range(1, NCORES):
        out += res.results[c]["out"]
    out += bo[None, :]
    return out.reshape(B, S, D)
